# revision 1
# baseline (speedup 1.0000x reference)
"""nn_CBMF kernel for Trainium2 — 8-core SPMD Bass implementation.

Reference computation (kmeans in the reference is dead code — its result
never reaches an output):
    u  = embed_user[user]          # [B, 128] gather from [1M, 128]
    it = embed_item[item]          # [B, 128] gather from [500K, 128]
    predict = 0.7*sum(u*it, -1) + average + user_bias[user] + item_bias[item]
    return (predict, u, it)

Distribution: batch-parallel — core c owns batch slice [c*16384, (c+1)*16384)
and holds replicated embedding tables; all gathers are core-local (no
collectives). Each table is augmented on host with its bias as a 129th
column so a single 516B indirect-DMA row-gather fetches embedding + bias.

Per core the Bass kernel issues 256 indirect DMA gathers (128 rows each, one
offset per partition — the HW ucode contract), streams the gathered tiles
back out to u_out/it_out via HWDGE, and computes the row-dot + bias/average
epilogue on the vector engine. Measured HW exec: ~393 us on NC-v3, which is
the SWDGE descriptor-generation floor (~8 ns/row on the Pool Q7) for the
262144 gathered rows; DMA payload itself (~33 MB/core) is far below the
bandwidth roofline.
"""
import numpy as np

import concourse.bass as bass
import concourse.mybir as mybir
from concourse import tile as _tile
from concourse.tile import TileContext
from concourse.vector_clock import ScopedClock

# ---------------------------------------------------------------------------
# Workaround: the walrus build in this toolchain rejects >1 sync-wait per
# instruction ("Too many sync wait commands" in setupSyncWait). Patch the
# TileContext exit to (a) split multi-wait instructions across single-wait
# same-engine nofuse nops inserted right before them, and (b) emit the final
# drain's global-clock waits on single-wait SP nops instead of the drain.
# ---------------------------------------------------------------------------


def _split_multiwait_insts(nc):
    for bb in nc.m.functions[0].blocks:
        insts = bb.instructions
        for inst in list(insts):
            si = inst.sync_info
            if si is None or not si.on_wait or len(si.on_wait) <= 1:
                continue
            waits = list(si.on_wait)
            keep = waits[-1]
            eng = nc.engines[inst.engine]
            idx = insts.index(inst)
            cur_insts = nc.cur_bb.bb.instructions
            for w in waits[:-1]:
                nop = eng.nop(hint="split_wait", nofuse=True).ins
                cur_insts.remove(nop)  # nop() appended it to cur_bb's tail
                nsi = nop.sync_info
                if nsi is None:
                    nop.sync_info = type(si)(on_wait=[w], on_update=[])
                else:
                    nsi.on_wait = [w]
                insts.insert(idx, nop)
                idx += 1
            si.on_wait = [keep]


def _patched_drain_and_barrier(self, tick_clock, wait_clock):
    carrier = self.nc.sync.nop(hint="pre_drain_waits", nofuse=True)
    wait_clock.add_sem_waits(
        carrier.ins, ScopedClock({None: tick_clock.global_clock})
    )
    si = carrier.ins.sync_info
    waits = list(si.on_wait or []) if si is not None else []
    if len(waits) > 1:
        si.on_wait = [waits[0]]
        for w in waits[1:]:
            extra = self.nc.sync.nop(hint="pre_drain_waits", nofuse=True)
            esi = extra.ins.sync_info
            if esi is None:
                extra.ins.sync_info = type(si)(on_wait=[w], on_update=[])
            else:
                esi.on_wait = list(esi.on_wait or []) + [w]
    _split_multiwait_insts(self.nc)
    self.nc.sync.drain()
    self.nc.all_engine_barrier()
    assert self.sems is not None
    popped = self.nc._tile_sem_poison_stack.pop()
    assert popped is self._sem_poison
    self.nc.clear_and_free_semaphores(list(self.sems.allocated().values()))
    self.nc.all_engine_barrier()


_tile.TileContext._drain_and_barrier = _patched_drain_and_barrier

# ---------------------------------------------------------------------------
# Problem shapes (hardcoded per spec)
# ---------------------------------------------------------------------------
USERNUM = 1_000_000
ITEMNUM = 500_000
D = 128
B = 131_072
NCORES = 8
BC = B // NCORES          # 16384 batch elements per core
P = 128                   # SBUF partitions
W = D + 1                 # augmented row: 128 embed floats + 1 bias float
CK = 16                   # index-tile columns per SBUF tile / write-back DMA


def _build_nc():
    """Per-core Bass program. Batch layout: b_local = p*COLS + c."""
    COLS = BC // P
    NCH = COLS // CK

    nc = bass.Bass()
    uidx = nc.dram_tensor("uidx", [BC], mybir.dt.int32, kind="ExternalInput")
    iidx = nc.dram_tensor("iidx", [BC], mybir.dt.int32, kind="ExternalInput")
    avg = nc.dram_tensor("avg_bc", [P, 1], mybir.dt.float32, kind="ExternalInput")
    eu = nc.dram_tensor("eu_aug", [USERNUM, W], mybir.dt.float32, kind="ExternalInput")
    ei = nc.dram_tensor("ei_aug", [ITEMNUM, W], mybir.dt.float32, kind="ExternalInput")
    pred = nc.dram_tensor("predict", [BC], mybir.dt.float32, kind="ExternalOutput")
    uo = nc.dram_tensor("u_out", [BC, D], mybir.dt.float32, kind="ExternalOutput")
    io = nc.dram_tensor("it_out", [BC, D], mybir.dt.float32, kind="ExternalOutput")

    with TileContext(nc) as tc:
        with (
            tc.tile_pool(name="misc", bufs=1) as misc,
            tc.tile_pool(name="gat", bufs=4) as gat,
            tc.tile_pool(name="prodp", bufs=3) as prodp,
        ):
            uidx_sb = misc.tile([P, COLS], mybir.dt.int32)
            iidx_sb = misc.tile([P, COLS], mybir.dt.int32)
            nc.sync.dma_start(out=uidx_sb[:], in_=uidx.rearrange("(p c) -> p c", p=P))
            nc.sync.dma_start(out=iidx_sb[:], in_=iidx.rearrange("(p c) -> p c", p=P))
            avg_bc = misc.tile([P, 1], mybir.dt.float32)
            nc.sync.dma_start(out=avg_bc[:], in_=avg[:, :])

            pred_sb = misc.tile([P, COLS], mybir.dt.float32)
            bsum_sb = misc.tile([P, COLS], mybir.dt.float32)
            uo_r = uo.rearrange("(p n j) d -> n p (j d)", p=P, j=CK)
            io_r = io.rearrange("(p n j) d -> n p (j d)", p=P, j=CK)
            for g in range(NCH):
                ut = gat.tile([P, CK * W], mybir.dt.float32, tag="ut")
                itt = gat.tile([P, CK * W], mybir.dt.float32, tag="itt")
                for j in range(CK):
                    c = g * CK + j
                    # one gather = 128 rows of 516B, one offset per partition
                    nc.gpsimd.indirect_dma_start(
                        out=ut[:, j * W:(j + 1) * W], out_offset=None, in_=eu[:, :],
                        in_offset=bass.IndirectOffsetOnAxis(
                            ap=uidx_sb[:, c:c + 1], axis=0),
                    )
                    nc.gpsimd.indirect_dma_start(
                        out=itt[:, j * W:(j + 1) * W], out_offset=None, in_=ei[:, :],
                        in_offset=bass.IndirectOffsetOnAxis(
                            ap=iidx_sb[:, c:c + 1], axis=0),
                    )
                ut3 = ut[:].rearrange("p (j w) -> p j w", w=W)
                it3 = itt[:].rearrange("p (j w) -> p j w", w=W)
                nc.sync.dma_start(out=uo_r[g], in_=ut3[:, :, :D])
                nc.sync.dma_start(out=io_r[g], in_=it3[:, :, :D])
                prod = prodp.tile([P, CK * D], mybir.dt.float32, tag="prod")
                nc.vector.tensor_mul(out=prod[:], in0=ut3[:, :, :D], in1=it3[:, :, :D])
                nc.vector.reduce_sum(
                    out=pred_sb[:, g * CK:(g + 1) * CK],
                    in_=prod[:].rearrange("p (j d) -> p j d", d=D),
                    axis=mybir.AxisListType.X,
                )
                nc.vector.tensor_add(
                    out=bsum_sb[:, g * CK:(g + 1) * CK],
                    in0=ut3[:, :, D:].rearrange("p j one -> p (j one)"),
                    in1=it3[:, :, D:].rearrange("p j one -> p (j one)"),
                )

            # predict = (dot*0.7 + average) + (ub + ib)
            nc.vector.tensor_scalar(
                out=pred_sb[:], in0=pred_sb[:],
                scalar1=0.7, scalar2=avg_bc[:, :1],
                op0=mybir.AluOpType.mult, op1=mybir.AluOpType.add,
            )
            nc.vector.tensor_add(out=pred_sb[:], in0=pred_sb[:], in1=bsum_sb[:])
            nc.sync.dma_start(out=pred.rearrange("(p c) -> p c", p=P), in_=pred_sb[:])
    return nc


def _augment(embed, bias):
    n, d = embed.shape
    out = np.empty((n, d + 1), dtype=np.float32)
    out[:, :d] = embed
    out[:, d] = bias
    return out


_NC_CACHE = None


def _run(inputs, trace=False):
    global _NC_CACHE
    from concourse.bass_utils import run_bass_kernel_spmd

    user = np.ascontiguousarray(np.asarray(inputs["user"]).astype(np.int32, copy=False))
    item = np.ascontiguousarray(np.asarray(inputs["item"]).astype(np.int32, copy=False))
    average = np.asarray(inputs["average"], dtype=np.float32).reshape(1)
    embed_user = np.asarray(inputs["embed_user"], dtype=np.float32)
    embed_item = np.asarray(inputs["embed_item"], dtype=np.float32)
    user_bias = np.asarray(inputs["user_bias"], dtype=np.float32)
    item_bias = np.asarray(inputs["item_bias"], dtype=np.float32)
    assert user.shape == (B,) and item.shape == (B,)
    assert embed_user.shape == (USERNUM, D) and embed_item.shape == (ITEMNUM, D)

    eu_aug = _augment(embed_user, user_bias)
    ei_aug = _augment(embed_item, item_bias)
    avg_bc = np.repeat(average, P).reshape(P, 1).astype(np.float32)

    if _NC_CACHE is None:
        _NC_CACHE = _build_nc()
    nc = _NC_CACHE

    in_maps = [
        {
            "uidx": user[c * BC:(c + 1) * BC],
            "iidx": item[c * BC:(c + 1) * BC],
            "avg_bc": avg_bc,
            "eu_aug": eu_aug,
            "ei_aug": ei_aug,
        }
        for c in range(NCORES)
    ]
    res = run_bass_kernel_spmd(nc, in_maps, list(range(NCORES)), trace=trace)
    predict = np.concatenate([res.results[c]["predict"] for c in range(NCORES)])
    u = np.concatenate([res.results[c]["u_out"] for c in range(NCORES)])
    it = np.concatenate([res.results[c]["it_out"] for c in range(NCORES)])
    return (predict, u, it), res


def kernel(**inputs):
    out, _ = _run(inputs, trace=False)
    return out
